# revision 18
# baseline (speedup 1.0000x reference)
"""Multi-head attention (B=8, N=2048, dim=64, heads=8) on 8 Trainium2 cores.

Sharding: batch-parallel - one batch element per NeuronCore, weights
replicated, no collectives. Per-core flash-style attention, fully
SBUF-resident.

v2: exp split across ACT/DVE/Pool engines (DVE+Pool via integer
Schraudolph exp-to-bf16-bits), fast reciprocal, bf16 output projection,
prefetch/setup/oproj share the st PSUM pool so PE never blocks on za.
"""
import sys

import numpy as np


def _ensure_path():
    try:
        import concourse  # noqa: F401
    except ImportError:
        for p in (
            "/opt/trn_rl_repo",
            "/root/.axon_site",
            "/root/.axon_site/_ro/trn_rl_repo",
            "/root/.axon_site/_ro/pypackages",
        ):
            if p not in sys.path:
                sys.path.append(p)


_ensure_path()

import concourse.bacc as bacc  # noqa: E402
import concourse.mybir as mybir  # noqa: E402
import concourse.tile as tile  # noqa: E402
from concourse.bass_utils import run_bass_kernel_spmd  # noqa: E402
from concourse.masks import make_identity  # noqa: E402

import os  # noqa: E402

DBG_NO_SCHRAUDOLPH = bool(os.environ.get("DBG_NO_SCHRAUDOLPH"))
DBG_SAFE_MISC = bool(os.environ.get("DBG_SAFE_MISC"))
DBG_SAFE_RECIP = DBG_SAFE_MISC or bool(os.environ.get("DBG_SAFE_RECIP"))
DBG_SAFE_MUL = DBG_SAFE_MISC or bool(os.environ.get("DBG_SAFE_MUL"))
DBG_SAFE_DMA = DBG_SAFE_MISC or bool(os.environ.get("DBG_SAFE_DMA"))

B, N, D, H = 8, 2048, 64, 8
P = 128
NT = N // P          # 16 j-tiles of 128
IC = N // 512        # 4 query chunks of 512
SCALE = float(D) ** -0.5
F32 = mybir.dt.float32
F32R = mybir.dt.float32r
BF16 = mybir.dt.bfloat16
I16 = mybir.dt.int16

# Schraudolph exp via bf16 bit pattern: bits = A16*s + B16.
# PSUM holds A16*s (Q pre-scaled by A16*SCALE); DVE/Pool add B16 and
# write int16 (truncation after +0.5 = round); ACT computes the exact
# exp(psum/A16). B16 includes -5.5 bits sawtooth centering so the
# mantissa-linear error is +-3% instead of [0, +6%].
A16 = 128.0 / float(np.log(2.0))        # 184.664
QPRE = A16 * SCALE                      # folded into W_q columns
EXPSCALE = 1.0 / A16
B16C = 16256.0 + 0.5 - 5.5

ALU = mybir.AluOpType
AF = mybir.ActivationFunctionType


def build_program(n_cores=B):
    nc = bacc.Bacc("TRN2", target_bir_lowering=False, debug=False,
                   num_devices=n_cores)
    x_d = nc.dram_tensor("x", [N, D], F32, kind="ExternalInput")
    wqkv_d = nc.dram_tensor("w_qkv", [D, 3 * H * D], F32, kind="ExternalInput")
    wout_d = nc.dram_tensor("w_out", [H * D, D], F32, kind="ExternalInput")
    bout_d = nc.dram_tensor("b_out", [D], F32, kind="ExternalInput")
    out_d = nc.dram_tensor("out", [N, D], F32, kind="ExternalOutput")

    with tile.TileContext(nc) as tc:
        with tc.tile_pool(name="const", bufs=1) as const:
            ident = const.tile([P, P], F32, tag="ident")
            make_identity(nc, ident[:])

            wsb = const.tile([D, 3 * H * D], F32R, tag="wqkv")
            nc.gpsimd.dma_start(wsb[:], wqkv_d.ap())
            # W_out in bf16 (cast during DMA on gpsimd)
            wout_sb = const.tile([P, 4, D], BF16, tag="wout")
            if DBG_SAFE_DMA:
                wout_f = const.tile([P, 4, D], F32, tag="woutf")
                nc.gpsimd.dma_start(
                    wout_f[:], wout_d.ap().rearrange("(t p) d -> p t d", p=P))
                nc.vector.tensor_copy(wout_sb[:], wout_f[:])
            else:
                nc.gpsimd.dma_start(
                    wout_sb[:],
                    wout_d.ap().rearrange("(t p) d -> p t d", p=P))
            b_row = const.tile([1, D], F32, tag="brow")
            nc.sync.dma_start(b_row[:], bout_d.ap().rearrange("(a d) -> a d", a=1))
            b_bc = const.tile([P, D], F32, tag="bbc")
            nc.gpsimd.partition_broadcast(b_bc[:], b_row[:])
            ones3 = const.tile([P, H, 1], BF16, tag="ones3")
            nc.gpsimd.memset(ones3[:], 1.0)
            # zeros operand for the stt clamp slot (op1 = max)
            zer = const.tile([P, 1024], BF16, tag="zer")
            nc.gpsimd.memset(zer[:], 0.0)

            xT = const.tile([D, N], F32R, tag="xT")
            # qk_sb[0..3]: Q^T head-pairs [128, N] (pre-scaled by A16*SCALE)
            # qk_sb[4..7]: K^T pairs
            qk_sb = [const.tile([P, N], BF16, tag=f"qk{i}", name=f"qk{i}")
                     for i in range(8)]
            # V~ per n-tile: [128, H, 65]; col 64 of each head is ones
            vt_sb = [const.tile([P, H, 65], BF16, tag=f"vt{t}", name=f"vt{t}")
                     for t in range(NT)]
            zT = [const.tile([P, N], BF16, tag=f"zT{i}", name=f"zT{i}")
                  for i in range(4)]

            with (
                tc.tile_pool(name="xin", bufs=1) as xpool,
                tc.tile_pool(name="spsum", bufs=3,
                             space=bacc.bass.MemorySpace.PSUM) as spsum,
                tc.tile_pool(name="zpsum", bufs=1,
                             space=bacc.bass.MemorySpace.PSUM) as zpsum,
                tc.tile_pool(name="es", bufs=4) as es_pool,
                tc.tile_pool(name="sm", bufs=2) as sm_pool,
                tc.tile_pool(name="outp", bufs=3) as outp,
            ):
                xall = xpool.tile([P, NT, D], F32, tag="xall")
                nc.sync.dma_start(
                    xall[:], x_d.ap().rearrange("(t p) d -> p t d", p=P))

                def st_tile():
                    # [128, 1024] = 2 PSUM banks; 3 bufs; shared by S-matmul
                    # chunks, setup, prefetch and oproj
                    return spsum.tile([P, 1024], F32, tag="st", name="st")

                def emit_qk(ct, icxs, copy_eng):
                    # Q tiles (ct < 4) get the A16*SCALE factor folded in
                    # during the PSUM->SBUF copy
                    w_sl = wsb[:, ct * P:(ct + 1) * P]
                    for icx in icxs:
                        mp = st_tile()
                        nc.tensor.matmul(
                            mp[0:P, 0:512], w_sl,
                            xT[:, icx * 512:(icx + 1) * 512],
                            start=True, stop=True)
                        dst = qk_sb[ct][:, icx * 512:(icx + 1) * 512]
                        if ct < 4:
                            copy_eng.scalar_tensor_tensor(
                                dst, mp[0:P, 0:512], QPRE, zer[:, 0:512],
                                op0=ALU.mult, op1=ALU.add)
                        else:
                            copy_eng.tensor_copy(dst, mp[0:P, 0:512])

                # ---- setup: transposes, Q/K for pair 0, all V
                for g in range(IC):
                    for t in range(4 * g, 4 * g + 4):
                        pp = st_tile()
                        nc.tensor.transpose(pp[0:D, 0:P], xall[:, t, :],
                                            ident[:])
                        nc.vector.tensor_copy(xT[:, t * P:(t + 1) * P],
                                              pp[0:D, 0:P])
                    emit_qk(4, [g], nc.vector)
                    emit_qk(0, [g], nc.vector)
                for t in range(NT):
                    mp = st_tile()
                    nc.tensor.matmul(
                        mp[0:P, 0:512], xT[:, t * P:(t + 1) * P],
                        wsb[:, 2 * H * D:3 * H * D],
                        start=True, stop=True)
                    nc.gpsimd.tensor_copy(vt_sb[t][:, :, 64:65], ones3[:])
                    nc.vector.tensor_copy(
                        vt_sb[t][:, :, 0:64],
                        mp[0:P, 0:512].rearrange("p (h d) -> p h d", h=H))

                # ---- main loop
                # chunk j covers both heads of a pair: st[:, 0:512] = head0,
                # st[:, 512:1024] = head1. exp split ACT 11 : DVE 5
                # (Pool cannot read PSUM).
                # AV runs AV_LAG chunks behind exp so the in-order PE queue
                # never blocks on an exp still in flight
                AV_LAG = 2
                pending = []  # [(es, j, za, hp), ...]

                def flush_av(nc, all_=False):
                    while pending and (all_ or len(pending) > AV_LAG):
                        es_p, j, za_p, hp_p = pending.pop(0)
                        for hh in (0, 1):
                            nc.tensor.matmul(
                                za_p[hh][:], vt_sb[j][:, 2 * hp_p + hh, :],
                                es_p[:, hh * 512:(hh + 1) * 512],
                                start=(j == 0), stop=(j == NT - 1),
                                skip_group_check=True)

                norm_pending = None  # (zus, hp, icx)

                def flush_norm(nc):
                    nonlocal norm_pending
                    if norm_pending is None:
                        return
                    zus_p, hp_p, icx_p = norm_pending
                    for hh in (0, 1):
                        rc = sm_pool.tile([1, 512], F32, tag="rc", name="rc")
                        if DBG_SAFE_RECIP:
                            nc.vector.reciprocal(rc[:], zus_p[hh][64:65, :])
                        else:
                            # 1/den = exp(-ln(den)) on ACT; both funcs live
                            # in the natural_log_exp_and_others table set
                            lnt = sm_pool.tile([1, 512], F32, tag="lnt",
                                               name="lnt")
                            nc.scalar.activation(lnt[:], zus_p[hh][64:65, :],
                                                 AF.Ln)
                            nc.scalar.activation(rc[:], lnt[:], AF.Exp,
                                                 scale=-1.0)
                        bc = sm_pool.tile([64, 512], F32, tag="bc", name="bc")
                        nc.gpsimd.partition_broadcast(bc[:], rc[:])
                        mul_eng = nc.vector if DBG_SAFE_MUL else nc.gpsimd
                        mul_eng.tensor_mul(
                            zT[hp_p][hh * 64:hh * 64 + 64,
                                     icx_p * 512:(icx_p + 1) * 512],
                            zus_p[hh][0:64, :], bc[:])
                    norm_pending = None

                def emit_oproj(tiles):
                    for t in tiles:
                        op = st_tile()
                        for ct in range(4):
                            nc.tensor.matmul(
                                op[0:P, 0:D], zT[ct][:, t * P:(t + 1) * P],
                                wout_sb[:, ct, :],
                                start=(ct == 0), stop=(ct == 3),
                                skip_group_check=True)
                        ot = outp.tile([P, D], F32, tag="ot", name="ot")
                        nc.vector.tensor_add(ot[:], op[0:P, 0:D], b_bc[:])
                        nc.sync.dma_start(out_d.ap()[t * P:(t + 1) * P, :],
                                          ot[:])

                for hp in range(H // 2):
                    for icx in range(IC):
                        # prefetch next head-pair's Q/K via the st pool
                        if hp + 1 < H // 2:
                            emit_qk(4 + hp + 1, [icx], nc.vector)
                            emit_qk(hp + 1, [icx], nc.vector)
                        flush_norm(nc)
                        qt = qk_sb[hp]
                        kt = qk_sb[4 + hp]
                        za = [zpsum.tile([65, 512], F32, tag="za0",
                                         name="za0", bufs=1),
                              zpsum.tile([65, 512], F32, tag="za1",
                                         name="za1", bufs=1)]
                        for j in range(NT):
                            st = st_tile()
                            es = es_pool.tile([P, 1024], BF16,
                                              tag="es", name="es")
                            for hh in (0, 1):
                                r0 = hh * 64
                                nc.tensor.matmul(
                                    st[:, hh * 512:(hh + 1) * 512],
                                    kt[r0:r0 + 64, j * P:(j + 1) * P],
                                    qt[r0:r0 + 64,
                                       icx * 512:(icx + 1) * 512],
                                    start=True, stop=True)
                            if ((j % 3 == 2 or j == 15)
                                    and not DBG_NO_SCHRAUDOLPH):
                                nc.vector.scalar_tensor_tensor(
                                    es[:].bitcast(I16), st[:], B16C,
                                    zer[:], op0=ALU.add, op1=ALU.max)
                            else:
                                nc.scalar.activation(
                                    es[:], st[:], AF.Exp, scale=EXPSCALE)
                            pending.append((es, j, za, hp))
                            flush_av(nc)
                        flush_av(nc, all_=True)
                        # stage za out of PSUM so banks free fast;
                        # normalization deferred one iteration
                        zus = []
                        for hh in (0, 1):
                            zu = sm_pool.tile([65, 512], F32, tag=f"zu{hh}",
                                              name=f"zu{hh}")
                            nc.vector.tensor_copy(zu[:], za[hh][:])
                            zus.append(zu)
                        norm_pending = (zus, hp, icx)
                flush_norm(nc)
                emit_oproj(range(NT))

    nc.compile()
    return nc


_PROG = None


def _get_program():
    global _PROG
    if _PROG is None:
        _PROG = build_program()
    return _PROG


def kernel(x, W_qkv, W_out, b_out):
    nc = _get_program()
    x = np.asarray(x, dtype=np.float32)
    wq = np.ascontiguousarray(np.asarray(W_qkv, dtype=np.float32))
    wo = np.ascontiguousarray(np.asarray(W_out, dtype=np.float32))
    bo = np.ascontiguousarray(np.asarray(b_out, dtype=np.float32))
    in_maps = [
        {"x": np.ascontiguousarray(x[i]), "w_qkv": wq, "w_out": wo,
         "b_out": bo}
        for i in range(B)
    ]
    res = run_bass_kernel_spmd(nc, in_maps, list(range(B)))
    return np.stack([res.results[i]["out"] for i in range(B)], axis=0)
